# revision 18
# baseline (speedup 1.0000x reference)
"""Trainium2 Bass kernel for nn_CategoryAlign_Module (pooling / cross Pearson).

Math (see reference):
  for each stream s in {1,2}:
    vec_b[k,c]  = sum_p preds[b,k,p] * feats[b,c,p] / sum_p preds[b,k,p]
    ctx_b[k,c]  = vec_b[k,c] / max(||vec_b[:,c]||_2, 1e-12)      (norm over K)
    ctx[k,c]    = mean_b ctx_b[k,c]
  out = pearson(ctx1, ctx2)   (center+normalize rows over C, then ctx1 @ ctx2^T)

Distribution: data-parallel over the batch dim, one batch element per
NeuronCore (B=8, 8 cores).  Each core computes its local normalized
contexts, the tiny [19,257] payloads are AllGather-ed across the 8
cores and summed locally (Pearson is invariant to the 1/B scale), and
every core redundantly computes the replicated [19,19] correlation.

Per-core pipeline (all big work in bf16, fp32 PSUM accumulate):
  - both preds and feats arrive host-relayouted to spatial-major bf16,
    so the kernel is pure streaming matmul: no on-chip transposes, no
    dtype-cast DMAs, and half the HBM traffic of an fp32 layout.
  - per-class mask sums come from cheap ones-vector matmuls over the
    (tiny, early-loaded) preds operand, fully hidden under the feats
    stream, so their reciprocals are ready before each stream's
    epilogue starts.
  - feats stream over the SP HWDGE ring in ~2 MB slabs; the PE chases
    the slabs with [128,19]^T @ [128,257] accumulating matmuls.
  - engine-queue discipline (queues are strict FIFO, so a DMA that
    waits on a collective must never be queued ahead of compute):
    SP ring carries the feats stream and, emitted after ALL slab
    issues, the post-collective readback + final output store; the
    ACT ring carries only payload bounce-out DMAs (which never wait
    on collectives); gpsimd carries the collective trigger.
  - ONE AllGather carries both streams' payloads in a 64-row padded
    block.  (Two early collectives lose badly here: the stream-1
    payload bounce crawls ~13 us under the feats DMA flood —
    packet-granularity round-robin against 16-engine slab traffic —
    which delays the first collective's trigger, and ncfw executes
    collectives strictly in order, serializing the second one after
    it.  AllGather beats AllReduce by ~6 us of ncfw setup latency at
    this size; the 64-row padding makes the gathered [512, 257] result
    readable as four contiguous [128, 257] DMAs, and the 8-rank sum is
    then four full-width DVE adds.)
  - after the AllReduce, BOTH sides' Pearson prep (center, 1/std,
    normalize) runs as single 38-partition ops, then 4 packed PE
    transposes and the final [19,19] matmuls.
"""

import sys

sys.path.insert(0, "/opt/trn_rl_repo")

import numpy as np

import concourse.bass as bass  # noqa: F401  (import order matters)
import concourse.bacc as bacc
import concourse.tile as tile
import concourse.mybir as mybir
from concourse import bass_utils, bass2jax  # noqa: F401

B, K, C, H, W = 8, 19, 256, 128, 128
P = H * W            # 16384 spatial positions
NCHUNK = P // 128    # 128 contraction chunks
PW = C + 1           # payload columns: 256 context + row-mean
PAD = 64             # payload rows padded to 64 for a clean gathered layout
SLABC = 16           # chunks per feats DMA slab (~1 MB bf16)
N_CORES = 8

F32 = mybir.dt.float32
BF16 = mybir.dt.bfloat16


def _slab_schedule(nchunk):
    """Per-stream list of (chunk0, nchunks) DMA slabs.

    Stream 1's tail is split into small slabs so the final
    DMA-then-matmul dependency at the very end of the pipeline is
    short.
    """
    slabs = []
    for s in (0, 1):
        sl, i = [], 0
        while i < nchunk:
            w = min(SLABC, nchunk - i)
            if s == 1 and i + w == nchunk and w == SLABC:
                for w2 in (8, 4, 4):
                    sl.append((i, w2))
                    i += w2
            else:
                sl.append((i, w))
                i += w
        slabs.append(sl)
    return slabs


def build_body(nc, tc, pret_d, featsr_d, identf_d, out_d, n_cores,
               nchunk=NCHUNK):
    """Emit the per-core program.

    pret_d:   2 DRAM APs [128, nchunk*K] bf16 (preds, spatial-major)
    featsr_d: 2 DRAM APs [128, nchunk*C] bf16 (feats, spatial-major)
    identf_d: [K, K] fp32 identity (for PE transposes)
    out_d:    [K, K] fp32 output
    """
    mult = mybir.AluOpType.mult
    RG = [list(range(n_cores))]
    slabs = _slab_schedule(nchunk)
    last_chunk = nchunk - 1

    with tc.tile_pool(name="persist", bufs=1) as PP, \
         tc.tile_pool(name="acc0", bufs=1, space="PSUM") as PA0, \
         tc.tile_pool(name="acc1", bufs=1, space="PSUM") as PA1, \
         tc.tile_pool(name="tailp", bufs=1, space="PSUM") as TLP, \
         tc.tile_pool(name="srp", bufs=1, space="PSUM") as SRP, \
         tc.tile_pool(name="dram", bufs=1, space="DRAM") as DP:

        # --- preds + constants on the ACT ring so the SP ring is
        # feats-only with no mid-stream holes ---
        id_f = PP.tile([K, K], F32, name="id_f")
        ones19 = PP.tile([K, 1], F32, name="ones19")
        nc.vector.memset(ones19[:], 1.0)
        onesrow = PP.tile([1, K], F32, name="onesrow")
        nc.vector.memset(onesrow[:], 1.0)
        ones_col = PP.tile([128, 1], BF16, name="ones_col")
        nc.vector.memset(ones_col[:], 1.0)

        # --- preds (spatial-major bf16), both streams up front ---
        PT = [PP.tile([128, nchunk * K], BF16, name=f"PT{s}") for s in (0, 1)]
        nc.scalar.dma_start(PT[0][:], pret_d[0][:])
        nc.scalar.dma_start(PT[1][:], pret_d[1][:])
        nc.scalar.dma_start(id_f[:], identf_d[:])

        psum_vec = [PA0.tile([K, C], F32, name="pvec0"),
                    PA1.tile([K, C], F32, name="pvec1")]
        # mask-sum rows, one [1, 8*K] accumulator per stream in one bank
        psum_srow = [SRP.tile([1, 8 * K], F32, name=f"psrow{s}")
                     for s in (0, 1)]
        # both streams' payloads, stacked on partitions [0:19] / [32:51]
        # (engine partition accesses must start 32-aligned).  The dead
        # rows are initialized so the batched tail math on them stays
        # finite: row=1.0, rowmean-col=0  ->  xc=8, ss>0, ri finite.
        pay = PP.tile([PAD, PW], BF16, name="pay")
        nc.vector.memset(pay[:], 1.0)
        nc.vector.memset(pay[:, C:C + 1], 0.0)
        b_in = DP.tile([PAD, PW], BF16, name="b_in")
        b_out = DP.tile([n_cores * PAD, PW], BF16, name="b_out")

        with tc.tile_pool(name="fslab", bufs=4) as FP, \
             tc.tile_pool(name="fslab_tail", bufs=3) as FT:
            for s in (0, 1):
                # mask sums: 16 accumulating ones-matmuls over the preds
                # operand (N=152 each), hidden under the feats stream;
                # group g adds chunks [8g, 8g+8) so lane (c, k) of the
                # row ends up holding sum over groups.
                ngrp = max(nchunk // 8, 1)
                gw = nchunk // ngrp
                for g in range(ngrp):
                    nc.tensor.matmul(
                        psum_srow[s][:],
                        lhsT=ones_col[:],
                        rhs=PT[s][:, g * gw * K:(g + 1) * gw * K],
                        start=(g == 0), stop=(g == ngrp - 1))
                for si, (c0, w) in enumerate(slabs[s]):
                    pool = FP if w == SLABC else FT
                    fsl = pool.tile([128, w * C], BF16, name="fsl")
                    nc.sync.dma_start(fsl[:, 0:w * C],
                                      featsr_d[s][:, c0 * C:(c0 + w) * C])
                    for t in range(w):
                        i = c0 + t
                        nc.tensor.matmul(
                            psum_vec[s][:],
                            lhsT=PT[s][:, i * K:(i + 1) * K],
                            rhs=fsl[:, t * C:(t + 1) * C],
                            start=(i == 0), stop=(i == last_chunk))

                # mask-sum reciprocals, ready well before the epilogue
                srow_sb = PP.tile([1, 8 * K], F32, name=f"srow{s}")
                nc.vector.tensor_copy(srow_sb[:], psum_srow[s][:])
                s19 = PP.tile([1, K], F32, name=f"s19_{s}")
                nc.vector.reduce_sum(
                    s19[:], srow_sb[:].rearrange("p (c k) -> p k c", c=gw),
                    axis=mybir.AxisListType.X)
                stmp = TLP.tile([K, 1], F32, name="stmp", tag="tlpB")
                nc.tensor.matmul(stmp[:], lhsT=s19[:], rhs=id_f[0:1, 0:1],
                                 is_transpose=True, start=True, stop=True)
                recip = PP.tile([K, 1], F32, name=f"recip{s}")
                nc.vector.reciprocal(recip[:], stmp[:])

                # ---- stream epilogue: local normalized context payload
                # into pay[s*K:(s+1)*K].  Mask sums arrived in PSUM
                # column 256 via the ones column.
                ps = pay[32 * s:32 * s + K, :]
                vec_sb = PP.tile([K, C], F32, name=f"vec_sb{s}")
                nc.vector.tensor_scalar_mul(vec_sb[:], psum_vec[s][:, 0:C],
                                            recip[:])
                sq = PP.tile([K, C], F32, name=f"sq{s}")
                nc.vector.tensor_mul(sq[:], vec_sb[:], vec_sb[:])
                # column sums over K via fp32 matmul with a ones vector
                pn = TLP.tile([1, C], F32, name="pn", tag="tlp")
                nc.tensor.matmul(pn[:], lhsT=ones19[:], rhs=sq[:],
                                 start=True, stop=True)
                # reference clamps the norm at 1e-12; the norm here is
                # O(1e-2) for non-degenerate input, so the clamp is a no-op.
                nsb = PP.tile([1, C], F32, name=f"nsb{s}")
                nc.scalar.sqrt(nsb[:], pn[:])
                rn = PP.tile([1, C], F32, name=f"rn{s}")
                nc.vector.reciprocal(rn[:], nsb[:])
                # broadcast 1/norm to the K partitions (rank-1 matmul)
                bc = TLP.tile([K, C], F32, name="bc", tag="tlp")
                nc.tensor.matmul(bc[:], lhsT=onesrow[:], rhs=rn[:],
                                 start=True, stop=True)
                nc.vector.tensor_mul(ps[:, 0:C], vec_sb[:], bc[:])
                # ship the per-core row-mean in the payload (mean over B
                # and mean over C commute)
                xdum = PP.tile([K, C], F32, name=f"xdum{s}")
                rm = PP.tile([K, 1], F32, name=f"rm{s}")
                nc.scalar.activation(xdum[:], ps[:, 0:C],
                                     mybir.ActivationFunctionType.Copy,
                                     scale=1.0 / C,
                                     accum_out=rm[:])
                nc.vector.tensor_copy(ps[:, C:C + 1], rm[:])
                # bounce this stream's payload half out; stream 0's DMA
                # may crawl under the feats flood, but it only gates the
                # (single, late) collective trigger, which also needs
                # stream 1's half anyway.
                nc.scalar.dma_start(b_in[32 * s:32 * s + K, :], ps[:])

            # ---- ONE AllGather for both streams' payloads ----
            nc.gpsimd.collective_compute(
                "AllGather", mybir.AluOpType.bypass,
                replica_groups=RG,
                ins=[b_in.opt()], outs=[b_out.opt()])

            # ---- tail, emitted AFTER every slab DMA issue (SP ring):
            # the gathered [512, 257] reads back as four contiguous
            # [128, 257] tiles (2 ranks each); the 8-rank sum is three
            # full-width adds plus one 64-partition-offset add. ----
            gt = PP.tile([128, 4 * PW], BF16, name="gt")
            nc.sync.dma_start(
                gt[:].rearrange("p (i c) -> p i c", i=4),
                b_out[:].rearrange("(i p) c -> p i c", i=4))
            g01 = PP.tile([128, PW], F32, name="g01")
            nc.vector.tensor_add(g01[:], gt[:, 0:PW], gt[:, PW:2 * PW])
            g23 = PP.tile([128, PW], F32, name="g23")
            nc.vector.tensor_add(g23[:], gt[:, 2 * PW:3 * PW],
                                 gt[:, 3 * PW:4 * PW])
            gs = PP.tile([128, PW], F32, name="gs")
            nc.vector.tensor_add(gs[:], g01[:], g23[:])
            # fold the two 64-row halves: two-SBUF-input ops need equal
            # base partitions, so stage the odd-rank half at base 0 first
            gso = PP.tile([32 + K, PW], F32, name="gso")
            nc.vector.tensor_copy(gso[:], gs[64:64 + 32 + K, :])
            X = PP.tile([32 + K, PW], F32, name="X")
            nc.vector.tensor_add(X[:], gs[0:32 + K, :], gso[:])
            xsq = PP.tile([32 + K, C], F32, name="xsq")
            ss = PP.tile([32 + K, 1], F32, name="ss")
            xc = PP.tile([32 + K, C], F32, name="xc")
            nc.vector.tensor_scalar_sub(xc[:], X[:, 0:C], X[:, C:C + 1])
            nc.scalar.activation(xsq[:], xc[:],
                                 mybir.ActivationFunctionType.Square,
                                 accum_out=ss[:])
            sd = PP.tile([32 + K, 1], F32, name="sd")
            nc.scalar.sqrt(sd[:], ss[:])
            ri = PP.tile([32 + K, 1], F32, name="ri")
            nc.vector.reciprocal(ri[:], sd[:])
            # normalize into two base-0 tiles (matmul lhsT requires base
            # partition 0/32/64, so the [19:38] slice can't feed the PE)
            xn = [PP.tile([K, C], F32, name=f"xn{s}") for s in (0, 1)]
            for s in (0, 1):
                nc.vector.tensor_scalar_mul(xn[s][:],
                                            xc[32 * s:32 * s + K, :],
                                            ri[32 * s:32 * s + K, :])
            # transpose both sides' [19, 256] into [128, 4*19] PSUM:
            # blocks (side, half) at columns (2*s + h) * K
            tps = TLP.tile([128, 4 * K], F32, name="tps", tag="tlpA")
            for s in (0, 1):
                for h in (0, 1):
                    j = 2 * s + h
                    nc.tensor.matmul(
                        tps[:, j * K:(j + 1) * K],
                        lhsT=xn[s][:, h * 128:(h + 1) * 128],
                        rhs=id_f[:], is_transpose=True,
                        start=(j == 0), stop=(j == 3))
            nT = PP.tile([128, 4 * K], F32, name="nT")
            nc.vector.tensor_copy(nT[:], tps[:])

            # ---- final correlation: contract the two 128-row halves ----
            po = TLP.tile([K, K], F32, name="po", tag="tlpC")
            for h in (0, 1):
                nc.tensor.matmul(po[:],
                                 lhsT=nT[:, h * K:(h + 1) * K],
                                 rhs=nT[:, (2 + h) * K:(3 + h) * K],
                                 start=(h == 0), stop=(h == 1))
            osb = PP.tile([K, K], F32, name="osb")
            nc.vector.tensor_copy(osb[:], po[:])
            nc.sync.dma_start(out_d[:], osb[:])


def build(n_cores=N_CORES, nchunk=NCHUNK):
    nc = bacc.Bacc("TRN2", target_bir_lowering=False, debug=False,
                   enable_asserts=False, num_devices=n_cores)
    pret_d = [nc.dram_tensor(f"pret{s}", [128, nchunk * K], BF16,
                             kind="ExternalInput").ap() for s in (1, 2)]
    featsr_d = [nc.dram_tensor(f"featsr{s}", [128, nchunk * C], BF16,
                               kind="ExternalInput").ap() for s in (1, 2)]
    identf_d = nc.dram_tensor("identf", [K, K], F32,
                              kind="ExternalInput").ap()
    out_d = nc.dram_tensor("out", [K, K], F32, kind="ExternalOutput").ap()
    with tile.TileContext(nc) as tc:
        build_body(nc, tc, pret_d, featsr_d, identf_d, out_d, n_cores,
                   nchunk=nchunk)
    nc.compile()
    return nc


_NC_CACHE = {}


def _get_nc():
    if "nc" not in _NC_CACHE:
        _NC_CACHE["nc"] = build(N_CORES)
    return _NC_CACHE["nc"]


class Runner:
    """Executes the compiled Bass program on the first `n_cores` jax
    devices via shard_map, with inputs pre-staged on the devices (the
    analog of the native path's input pre-load in run_neff) so all
    cores start the NEFF near-simultaneously."""

    def __init__(self, nc, n_cores):
        import jax
        from jax.experimental.shard_map import shard_map
        from jax.sharding import Mesh, PartitionSpec, NamedSharding

        bass2jax.install_neuronx_cc_hook()
        self.jax = jax
        self.nc = nc
        self.n_cores = n_cores
        assert nc.dbg_addr is None
        partition_name = (nc.partition_id_tensor.name
                          if nc.partition_id_tensor else None)
        in_names, out_names, out_avals = [], [], []
        for alloc in nc.m.functions[0].allocations:
            if not isinstance(alloc, mybir.MemoryLocationSet):
                continue
            name = alloc.memorylocations[0].name
            if alloc.kind == "ExternalInput":
                if name != partition_name:
                    in_names.append(name)
            elif alloc.kind == "ExternalOutput":
                shape = tuple(alloc.tensor_shape)
                dtype = mybir.dt.np(alloc.dtype)
                out_names.append(name)
                out_avals.append(jax.core.ShapedArray(shape, dtype))
        self.param_names = list(in_names)
        n_params = len(in_names)
        full_in_names = list(in_names) + list(out_names)
        if partition_name is not None:
            full_in_names.append(partition_name)
        full_in_names = tuple(full_in_names)
        donate = tuple(range(n_params, n_params + len(out_names)))
        self.out_names = out_names
        self.out_avals = out_avals

        def _body(*args):
            operands = list(args)
            if partition_name is not None:
                operands.append(bass2jax.partition_id_tensor())
            outs = bass2jax._bass_exec_p.bind(
                *operands,
                out_avals=tuple(out_avals),
                in_names=full_in_names,
                out_names=tuple(out_names),
                lowering_input_output_aliases=(),
                sim_require_finite=True,
                sim_require_nnan=True,
                nc=nc,
            )
            return tuple(outs)

        devices = jax.devices()[:n_cores]
        assert len(devices) == n_cores
        self.mesh = Mesh(np.asarray(devices), ("core",))
        in_specs = (PartitionSpec("core"),) * (n_params + len(out_names))
        out_specs = (PartitionSpec("core"),) * len(out_names)
        self.fn = jax.jit(
            shard_map(_body, mesh=self.mesh, in_specs=in_specs,
                      out_specs=out_specs, check_rep=False),
            donate_argnums=donate, keep_unused=True)
        self.sharding = NamedSharding(self.mesh, PartitionSpec("core"))

    def put(self, in_maps):
        concat = [
            np.concatenate([np.asarray(in_maps[c][n])
                            for c in range(self.n_cores)], axis=0)
            for n in self.param_names
        ]
        arrs = [self.jax.device_put(a, self.sharding) for a in concat]
        self.jax.block_until_ready(arrs)
        return arrs

    def zeros(self):
        zs = [self.jax.device_put(
            np.zeros((self.n_cores * a.shape[0], *a.shape[1:]), a.dtype),
            self.sharding) for a in self.out_avals]
        self.jax.block_until_ready(zs)
        return zs

    def exec(self, dev_in):
        outs = self.fn(*dev_in, *self.zeros())
        self.jax.block_until_ready(outs)
        return {
            name: np.asarray(outs[i]).reshape(
                self.n_cores, *self.out_avals[i].shape)
            for i, name in enumerate(self.out_names)
        }


def _get_runner():
    if "runner" not in _NC_CACHE:
        _NC_CACHE["runner"] = Runner(_get_nc(), N_CORES)
    return _NC_CACHE["runner"]


def make_in_maps(preds1, feats1, preds2, feats2):
    import ml_dtypes
    identf = np.eye(K, dtype=np.float32)
    # feats [B, C, H, W] -> [B, W(p), H(i), C] bf16:
    # FS[p, i*256 + c] = feats[c, i*128 + p]
    fa = []
    for f in (feats1, feats2):
        a = np.empty((B, 128, NCHUNK, C), dtype=ml_dtypes.bfloat16)
        a[...] = f.transpose(0, 3, 2, 1)
        fa.append(a.reshape(B, 128, NCHUNK * C))
    # preds [B, K, H, W] -> [B, W(p), H(i), K]: chunk i's columns are
    # P^T[i*128:(i+1)*128, :] with the spatial index on partitions
    pt = [p.transpose(0, 3, 2, 1).astype(ml_dtypes.bfloat16).reshape(
              B, 128, NCHUNK * K) for p in (preds1, preds2)]
    in_maps = []
    for b in range(B):
        in_maps.append({
            "pret1": pt[0][b],
            "pret2": pt[1][b],
            "featsr1": fa[0][b],
            "featsr2": fa[1][b],
            "identf": identf,
        })
    return in_maps


def kernel(preds1, feats1, preds2, feats2):
    runner = _get_runner()
    in_maps = make_in_maps(preds1, feats1, preds2, feats2)
    dev_in = runner.put(in_maps)
    outs = runner.exec(dev_in)
    return np.asarray(outs["out"][0], dtype=np.float32)
